# revision 18
# baseline (speedup 1.0000x reference)
"""Trainium2 Bass kernel for DissimilarityMixtureEncoderCov forward.

Computes softmax(-ALPHA * D + log(relu(mixers)), axis=-1) where
  D[b,k] = (x_b - mu_k)^T (C_k C_k^T) (x_b - mu_k)

Data-parallel over batch across 8 NeuronCores. Per core:
  Y[b,(k,j)] = x_b . C_k[:,j]          (fp32 matmul, contraction d=128)
  T1[b,k]    = ALPHA * sum_j Y^2       (ACT square(scale) + DVE grouped reduce)
  t_k        = C_k^T mu_k              (DVE broadcast-mult + reduce, interleaved)
  v_k        = C_k t_k                 (DVE broadcast-mult + reduce, interleaved)
  logits     = -T1 + 2a*x.v + (-a*||t_k||^2 + log(mixers))  (PE accumulation)
  out        = softmax(logits)
using the identity D = ||C^T x - C^T mu||^2 expanded in three terms.
"""

import sys

sys.path.insert(0, "/opt/trn_rl_repo")

import numpy as np

import concourse.bacc as bacc
import concourse.tile as tile
from concourse import mybir

ALPHA = 10.0
B, K, D = 8192, 128, 128
N_CORES = 8
B_LOC = B // N_CORES          # 1024 batch rows per core
N_CHUNKS = B_LOC // 128       # 8 chunks of 128 rows
KJ = K * D                    # 16384 columns of the big matmul
BLK = 2048                    # psum block = 4 banks (4 matmuls of 512)
N_BLK = KJ // BLK             # 16 blocks
SQRT_A = float(np.sqrt(ALPHA))

FP32 = mybir.dt.float32


def _build_bass():
    nc = bacc.Bacc("TRN2", target_bir_lowering=False, debug=False,
                   num_devices=N_CORES)

    x_d = nc.dram_tensor("x", [B_LOC, D], FP32, kind="ExternalInput")
    cov_d = nc.dram_tensor("cov", [K * D, D], FP32, kind="ExternalInput")
    cen_d = nc.dram_tensor("centers", [K, D], FP32, kind="ExternalInput")
    mix_d = nc.dram_tensor("mixers", [1, K], FP32, kind="ExternalInput")
    ident_d = nc.dram_tensor("ident", [128, 128], FP32, kind="ExternalInput")
    out_d = nc.dram_tensor("out", [B_LOC, K], FP32, kind="ExternalOutput")

    AF = mybir.ActivationFunctionType
    OP = mybir.AluOpType
    AX = mybir.AxisListType

    with tile.TileContext(nc) as tc:
        with (
            tc.tile_pool(name="const", bufs=1) as constp,
            tc.tile_pool(name="cov", bufs=1) as covp,
            tc.tile_pool(name="covk", bufs=1) as covkp,
            tc.tile_pool(name="prod", bufs=2) as prodp,
            tc.tile_pool(name="xt", bufs=1) as xtp,
            tc.tile_pool(name="small", bufs=1) as smallp,
            tc.tile_pool(name="work", bufs=3) as workp,
            tc.tile_pool(name="ysq", bufs=4) as ysqp,
            tc.tile_pool(name="py", bufs=2, space="PSUM") as pyp,
        ):
            # ---------- x first (unblocks PE transposes + main matmuls) ----
            x_sb = smallp.tile([128, N_CHUNKS * 128], FP32)  # [b, (c,d)]
            nc.sync.dma_start(
                out=x_sb[:, :].rearrange("b (c d) -> b c d", d=128),
                in_=x_d[:, :].rearrange("(c b) d -> b c d", b=128),
            )
            ident = constp.tile([128, 128], FP32)
            nc.sync.dma_start(out=ident[:, :], in_=ident_d[:, :])
            cen_sb = smallp.tile([128, 128], FP32)   # [k, d]
            nc.sync.dma_start(out=cen_sb[:, :], in_=cen_d[:, :])
            mix = smallp.tile([1, K], FP32)
            nc.sync.dma_start(out=mix[:, :], in_=mix_d[:, :])

            # ---------- cov in both layouts ----------
            # cov_g[g][d, g16*128+j] = cov[k, d, j] for k in group of 16
            G = 16
            cov_g = []
            for g0 in range(0, K, G):
                cg = covp.tile([128, G * 128], FP32, tag=f"cov{g0}")
                nc.sync.dma_start(
                    out=cg[:, :].rearrange("d (g j) -> d g j", j=128),
                    in_=cov_d[g0 * 128:(g0 + G) * 128, :].rearrange(
                        "(g d) j -> d g j", d=128),
                )
                cov_g.append(cg)
            # covk_sb[k, d*128+j] = cov[k, d, j]
            covk_sb = covkp.tile([128, KJ], FP32)
            nc.sync.dma_start(
                out=covk_sb[:, :].rearrange("k (d j) -> k d j", j=128),
                in_=cov_d[:, :].rearrange("(k d) j -> k d j", d=128),
            )

            # ---------- transpose x ----------
            xt_sb = xtp.tile([128, B_LOC], FP32)            # [d, b]
            for c in range(N_CHUNKS):
                tp = pyp.tile([128, 128], FP32, tag="py")
                nc.tensor.transpose(tp[:, :], x_sb[:, c * 128:(c + 1) * 128],
                                    ident[:, :])
                nc.scalar.copy(xt_sb[:, c * 128:(c + 1) * 128], tp[:, :])

            # small helpers
            ones_row = constp.tile([1, 128], FP32)
            nc.vector.memset(ones_row[:, :], 1.0)
            ones_col = constp.tile([128, 1], FP32)
            nc.vector.memset(ones_col[:, :], 1.0)
            bias_row = smallp.tile([1, K], FP32)
            nc.vector.tensor_scalar_max(bias_row[:, :], mix[:, :], 0.0)
            nc.scalar.activation(bias_row[:, :], bias_row[:, :], AF.Ln)

            t_sb = smallp.tile([128, 128], FP32)     # [k, j]
            v_sb = smallp.tile([128, 128], FP32)     # [k, d]
            JG = 16                                  # t computed in 8 j-slices
            DG = 16                                  # v computed in 8 d-slices
            cen_bc = cen_sb[:, :].rearrange(
                "k (d o) -> k d o", o=1).broadcast_to([128, 128, JG])
            t_bc = t_sb[:, :].rearrange(
                "k (o j) -> k o j", o=1).broadcast_to([128, DG, 128])

            def t_slice(i):
                # t[k, j0:j0+JG] = sum_d covk[k, (d,j)] * centers[k, d]
                j0 = i * JG
                prod = prodp.tile([128, 128 * JG], FP32, tag="prod")
                nc.vector.tensor_tensor(
                    out=prod[:, :].rearrange("k (d j) -> k d j", j=JG),
                    in0=covk_sb[:, :].rearrange(
                        "k (d j) -> k d j", j=128)[:, :, j0:j0 + JG],
                    in1=cen_bc, op=OP.mult)
                nc.vector.tensor_reduce(
                    out=t_sb[:, j0:j0 + JG],
                    in_=prod[:, :].rearrange("k (d j) -> k j d", j=JG),
                    axis=AX.X, op=OP.add)

            def v_slice(i):
                # v[k, d0:d0+DG] = sum_j covk[k, (d,j)] * t[k, j]
                d0 = i * DG
                prod = prodp.tile([128, DG * 128], FP32, tag="prod")
                nc.vector.tensor_tensor(
                    out=prod[:, :].rearrange("k (d j) -> k d j", j=128),
                    in0=covk_sb[:, d0 * 128:(d0 + DG) * 128].rearrange(
                        "k (d j) -> k d j", j=128),
                    in1=t_bc, op=OP.mult)
                nc.vector.tensor_reduce(
                    out=v_sb[:, d0:d0 + DG],
                    in_=prod[:, :].rearrange("k (d j) -> k d j", j=128),
                    axis=AX.X, op=OP.add)

            # ---------- phase 1: Y matmuls + square + grouped reduce,
            # with t/v slices interleaved into DVE slack ----------
            t1a_all = []
            for c in range(N_CHUNKS):
                lhsT = xt_sb[:, c * 128:(c + 1) * 128]
                t1a = workp.tile([128, K], FP32, tag=f"t1a{c}")
                t1a_all.append(t1a)
                for blk in range(N_BLK):
                    py = pyp.tile([128, BLK], FP32, tag="py")
                    for m in range(BLK // 512):
                        c0 = blk * BLK + m * 512
                        g = c0 // (G * 128)
                        o = c0 - g * G * 128
                        nc.tensor.matmul(py[:, m * 512:(m + 1) * 512], lhsT,
                                         cov_g[g][:, o:o + 512],
                                         start=True, stop=True)
                    ysq = ysqp.tile([128, BLK], FP32, tag="ysq")
                    nc.scalar.activation(ysq[:, :], py[:, :], AF.Square,
                                         scale=SQRT_A)
                    ng = BLK // 128
                    nc.vector.tensor_reduce(
                        out=t1a[:, blk * ng:(blk + 1) * ng],
                        in_=ysq[:, :].rearrange("b (g j) -> b g j", j=128),
                        axis=AX.X, op=OP.add)
                    # interleave one t/v slice twice per chunk
                    if blk % 4 == 1:
                        i = c * 2 + (blk // 4)
                        if i < 8:
                            t_slice(i)
                        else:
                            v_slice(i - 8)

            # vt2a[d, k] = 2*ALPHA * v[k, d]^T
            vt2a_sb = smallp.tile([128, 128], FP32)
            tpv = pyp.tile([128, 128], FP32, tag="py")
            nc.tensor.transpose(tpv[:, :], v_sb[:, :], ident[:, :])
            nc.scalar.activation(vt2a_sb[:, :], tpv[:, :], AF.Copy,
                                 scale=2.0 * ALPHA)

            # const row: -ALPHA*||t_k||^2 + bias_k
            tsq = smallp.tile([128, 128], FP32)
            nc.scalar.activation(tsq[:, :], t_sb[:, :], AF.Square)
            tsqt_p = pyp.tile([128, 128], FP32, tag="py")
            nc.tensor.transpose(tsqt_p[:, :], tsq[:, :], ident[:, :])
            tsqt = smallp.tile([128, 128], FP32)     # [j, k]
            nc.scalar.copy(tsqt[:, :], tsqt_p[:, :])
            crow_p = pyp.tile([1, 128], FP32, tag="py")
            nc.tensor.matmul(crow_p[:, :], ones_col[:, :], tsqt[:, :],
                             start=True, stop=True)
            const_row = smallp.tile([1, K], FP32)
            nc.scalar.activation(const_row[:, :], crow_p[:, :], AF.Copy,
                                 scale=-ALPHA)
            nc.vector.tensor_tensor(out=const_row[:, :], in0=const_row[:, :],
                                    in1=bias_row[:, :], op=OP.add)

            # ---------- phase 2: logits + softmax ----------
            for c in range(N_CHUNKS):
                lhsT = xt_sb[:, c * 128:(c + 1) * 128]
                t1a = t1a_all[c]

                pl = pyp.tile([128, K], FP32, tag="py")
                nc.tensor.matmul(pl[:, :], lhsT, vt2a_sb[:, :],
                                 start=True, stop=False)
                nc.tensor.matmul(pl[:, :], ones_row[:, :], const_row[:, :],
                                 start=False, stop=True)

                lg = workp.tile([128, K], FP32, tag="lg")
                nc.vector.tensor_tensor(out=lg[:, :], in0=pl[:, :],
                                        in1=t1a[:, :], op=OP.subtract)
                mx = workp.tile([128, 1], FP32, tag="mx")
                nc.vector.tensor_reduce(out=mx[:, :], in_=lg[:, :],
                                        axis=AX.X, op=OP.max)
                nmx = workp.tile([128, 1], FP32, tag="nmx")
                nc.vector.tensor_scalar_mul(nmx[:, :], mx[:, :], -1.0)
                ex = workp.tile([128, K], FP32, tag="ex")
                den = workp.tile([128, 1], FP32, tag="den")
                nc.scalar.activation(ex[:, :], lg[:, :], AF.Exp,
                                     bias=nmx[:, 0:1], accum_out=den[:, 0:1])
                rden = workp.tile([128, 1], FP32, tag="rden")
                nc.vector.reciprocal(rden[:, :], den[:, :])
                ot = workp.tile([128, K], FP32, tag="ot")
                nc.vector.tensor_scalar(out=ot[:, :], in0=ex[:, :],
                                        scalar1=rden[:, 0:1], scalar2=None,
                                        op0=OP.mult)
                nc.sync.dma_start(out=out_d[c * 128:(c + 1) * 128, :],
                                  in_=ot[:, :])

    nc.compile()
    return nc


_NC_CACHE = None


def kernel(x, centers, cov, mixers):
    global _NC_CACHE
    from concourse.bass_utils import run_bass_kernel_spmd

    if _NC_CACHE is None:
        _NC_CACHE = _build_bass()
    nc = _NC_CACHE

    x = np.ascontiguousarray(x, dtype=np.float32)
    cov2 = np.ascontiguousarray(cov, dtype=np.float32).reshape(K * D, D)
    cen = np.ascontiguousarray(centers, dtype=np.float32)
    mix = np.ascontiguousarray(mixers, dtype=np.float32)
    ident = np.eye(128, dtype=np.float32)

    in_maps = []
    for c in range(N_CORES):
        in_maps.append({
            "x": x[c * B_LOC:(c + 1) * B_LOC],
            "cov": cov2,
            "centers": cen,
            "mixers": mix,
            "ident": ident,
        })
    res = run_bass_kernel_spmd(nc, in_maps, list(range(N_CORES)))
    out = np.concatenate([res.results[c]["out"] for c in range(N_CORES)],
                         axis=0)
    return out
